# revision 4
# baseline (speedup 1.0000x reference)
"""ConvCaps (shared 3x3 conv + dynamic routing) Trainium2 Bass kernel — v2.

Sharding: data-parallel over batch B=8 -> 8 NeuronCores (1 image/core).

v2 design ("spatial-on-partitions" routing layout), sized for
B,Ci,Pi,Co,Po,K,H,W = 8,8,16,16,16,3,64,64:

  The image is processed in 4 quarters of 1024 pixels (16 rows). Per quarter:

  conv:  im2col patches pa[(tap,pi)=128, (ci,s)=8192] built by 9 shifted
         SBUF->SBUF DMAs from a host-prepared padded input stored as
         xpad_t[(pi,ci)=128, 66*66] (one DMA covers all ci). Votes are
         computed in the standard layout Vt[oc_slab=128, (ci,s)] with
         2 K-tile fp32 matmuls (taps0-7: K=128, tap8: K=16; fp32 avoids
         the separate LDWEIGHTS instruction of the 16-bit path) per
         512-chunk, N=512, accumulating in [128,2048] PSUM tiles,
         evacuated by ScalarE copies (fp32->bf16).

  transpose: one InstDmaTransposeAnt per (quarter, oc-slab) turns
         Vt[oc,(ci,s)] into the routing layout V[p=s%128, (ci, c=s//128,
         oc)] (bf16, 2-byte dtype required by the xbar transpose).

  routing: with spatial on partitions, co/po/ci are all free dims, so
         every reduction/broadcast of the dynamic-routing loop is a
         single big DVE/ACT instruction (tensor_reduce over the
         innermost dim / stride-0 broadcast APs). No matmuls, no
         partition ops:
           sj0   = reduce_ci(V)/16 + bias
           squash: r=reduce_po(sj^2); f=sqrt(r)/(1+r); vj=sj*f
           b-upd: e *= exp(reduce_po(V*vj))
           cij   = e / reduce_co(e)
           sj    = reduce_ci(V*cij) + bias
"""

import sys

sys.path.insert(0, "/opt/trn_rl_repo")

import numpy as np
import ml_dtypes

import concourse.bacc as bacc
import concourse.mybir as mybir
import concourse.tile as tile
from concourse import bass_utils

F32 = mybir.dt.float32
BF16 = mybir.dt.bfloat16
ALU = mybir.AluOpType
ACTF = mybir.ActivationFunctionType
AXX = mybir.AxisListType.X

B, CI, PI, CO, PO, KK = 8, 8, 16, 16, 16, 3
H = W = 64
SP = H * W            # 4096 spatial positions per image
NCORES = 8
HP, WP = H + 2, W + 2  # padded 66x66
NQ = 4                 # quarters per image
SQ = SP // NQ          # 1024 pixels per quarter
RQ = SQ // W           # 16 image rows per quarter
NC_ = SQ // 128        # 8 chunks of 128 pixels per quarter
OC = CO * PO           # 256


def _host_consts(conv_w: np.ndarray, biases: np.ndarray):
    cw = np.asarray(conv_w, np.float32)                       # [256, 16, 3, 3]
    wT = cw.transpose(2, 3, 1, 0).reshape(9 * PI, OC)         # [(tap,pi), oc]
    wTa = wT[:128].astype(np.float32)                         # taps 0-7
    wTb = np.ascontiguousarray(wT[128:]).astype(np.float32)   # tap 8
    bias_rep = np.tile(np.asarray(biases, np.float32).reshape(1, OC), (128, 1))
    return dict(wTa=np.ascontiguousarray(wTa), wTb=wTb,
                bias_rep=np.ascontiguousarray(bias_rep))


def _host_x(x: np.ndarray):
    """xpad_t per core: [(pi,ci)=128, 66*66] bf16."""
    xf = np.asarray(x, np.float32)                            # [B, Ci, Pi, H, W]
    xpad = np.zeros((B, CI, PI, HP, WP), np.float32)
    xpad[:, :, :, 1:H + 1, 1:W + 1] = xf
    xt = xpad.transpose(0, 2, 1, 3, 4).reshape(B, 128, HP * WP)  # (pi, ci)
    return np.ascontiguousarray(xt.astype(np.float32))


def _build(nc, routings: int, reps: int):
    dram = {}
    for name, shape, dt in [
        ("xpad_t", [128, HP * WP], F32),
        ("wTa", [128, OC], F32), ("wTb", [16, OC], F32),
        ("bias_rep", [128, OC], F32),
    ]:
        dram[name] = nc.dram_tensor(name, shape, dt, kind="ExternalInput")
    # out[p, (q, c, co, po)] : spatial s = q*1024 + c*128 + p
    out_d = nc.dram_tensor("out", [128, NQ * NC_ * OC], F32,
                           kind="ExternalOutput")

    with tile.TileContext(nc) as tc:
        with (
            tc.tile_pool(name="const", bufs=1) as cpool,
            tc.tile_pool(name="work", bufs=1) as wk,
            tc.tile_pool(name="stage", bufs=1) as stg,
            tc.tile_pool(name="ps", bufs=2, space="PSUM") as psp,
        ):
            xpad_sb = cpool.tile([128, HP * WP], F32, tag="xpad", name="xpad")
            wTa = cpool.tile([128, OC], F32, tag="wTa", name="wTa")
            wTb = cpool.tile([16, OC], F32, tag="wTb", name="wTb")
            bias_rep = cpool.tile([128, OC], F32, tag="bias", name="bias")
            for nm, t in [("xpad_t", xpad_sb), ("wTa", wTa), ("wTb", wTb),
                          ("bias_rep", bias_rep)]:
                nc.sync.dma_start(t[:], dram[nm].ap())
            xpad3 = xpad_sb[:].rearrange("p (h w) -> p h w", h=HP)

            for _rep in range(reps):
                for q in range(NQ):
                    r0 = q * RQ

                    # ---- im2col patches: one DMA per tap ----
                    pa = stg.tile([128, CI * SQ], F32, tag="pa", name="pa")
                    pb = stg.tile([16, CI * SQ], F32, tag="pb", name="pb")
                    for tap in range(9):
                        dh, dw = tap // 3, tap % 3
                        src = xpad3[:, r0 + dh:r0 + dh + RQ, dw:dw + W]
                        if tap < 8:
                            dst = pa[tap * 16:(tap + 1) * 16].rearrange(
                                "p (ci r c) -> p ci r c", ci=CI, r=RQ)
                        else:
                            dst = pb[:].rearrange(
                                "p (ci r c) -> p ci r c", ci=CI, r=RQ)
                        nc.sync.dma_start(dst, src)

                    # ---- conv -> Vt[oc_slab, (ci,s)] -> transpose to V ----
                    # V free layout: (ci 8, c 8, oc 256)
                    V = wk.tile([128, CI * NC_ * OC], BF16, tag="V", name="V")
                    for slab in range(2):
                        vt = stg.tile([128, CI * SQ], BF16, tag="vt", name="vt")
                        for g in range(4):
                            ps = psp.tile([128, 2048], F32, tag="cps", name="cps")
                            for cc in range(4):
                                ch = g * 4 + cc
                                nc.tensor.matmul(
                                    ps[:, cc * 512:(cc + 1) * 512],
                                    wTa[:, slab * 128:(slab + 1) * 128],
                                    pa[:, ch * 512:(ch + 1) * 512],
                                    start=True, stop=False)
                                nc.tensor.matmul(
                                    ps[:, cc * 512:(cc + 1) * 512],
                                    wTb[:, slab * 128:(slab + 1) * 128],
                                    pb[:, ch * 512:(ch + 1) * 512],
                                    start=False, stop=True)
                            nc.scalar.copy(vt[:, g * 2048:(g + 1) * 2048], ps[:])
                        # rows r=(ci,s_local): k=r//128=(ci,c), p=s_local%128
                        nc.sync.dma_start_transpose(
                            V[:].rearrange("p (k oc) -> p k oc", oc=OC)
                               [:, :, slab * 128:(slab + 1) * 128],
                            vt[:])

                    # ---- routing (all free-dim ops) ----
                    sj = wk.tile([128, NC_ * OC], F32, tag="sj", name="sj")
                    vjb = wk.tile([128, NC_ * OC], BF16, tag="vjb", name="vjb")
                    e = wk.tile([128, CI * NC_ * CO], F32, tag="e", name="e")
                    cij = wk.tile([128, CI * NC_ * CO], BF16, tag="cij", name="cij")
                    sq = wk.tile([128, NC_ * OC], F32, tag="sq", name="sq")
                    upd = wk.tile([128, CI * NC_ * CO], F32, tag="upd", name="upd")
                    et = wk.tile([128, CI * NC_ * CO], F32, tag="et", name="et")
                    r_ = wk.tile([128, NC_ * CO], F32, tag="r_", name="r_")
                    w_ = wk.tile([128, NC_ * CO], F32, tag="w_", name="w_")
                    f_ = wk.tile([128, NC_ * CO], F32, tag="f_", name="f_")
                    D_ = wk.tile([128, CI * NC_], F32, tag="D_", name="D_")
                    rD = wk.tile([128, CI * NC_], F32, tag="rD", name="rD")

                    bias_b = bias_rep[:].rearrange(
                        "p (c m) -> p c m", c=1).to_broadcast((128, NC_, OC))
                    sj3 = sj[:].rearrange("p (c m) -> p c m", c=NC_)

                    for t in range(routings):
                        if t == 0:
                            # sj0 = reduce_ci(V)/16 + bias
                            nc.vector.tensor_reduce(
                                sj[:],
                                V[:].rearrange("p (ci m) -> p m ci", ci=CI),
                                axis=AXX, op=ALU.add)
                            nc.vector.scalar_tensor_tensor(
                                out=sj3, in0=sj3, scalar=1.0 / CO,
                                in1=bias_b, op0=ALU.mult, op1=ALU.add)
                        else:
                            # sj = reduce_ci(V * cij_bc_po) + bias
                            Wk = wk.tile([128, CI * NC_ * OC], BF16,
                                         tag="scr", name="Wk")
                            nc.vector.tensor_mul(
                                Wk[:].rearrange("p (g co po) -> p g co po",
                                                co=CO, po=PO),
                                V[:].rearrange("p (g co po) -> p g co po",
                                               co=CO, po=PO),
                                cij[:].rearrange("p (g co x) -> p g co x", co=CO,
                                                 x=1).to_broadcast(
                                    (128, CI * NC_, CO, PO)))
                            nc.vector.tensor_reduce(
                                sj[:],
                                Wk[:].rearrange("p (ci m) -> p m ci", ci=CI),
                                axis=AXX, op=ALU.add)
                            nc.vector.tensor_add(sj3, sj3, bias_b)

                        # squash: f = sqrt(r)/(1+r), vj = sj*f
                        nc.vector.tensor_mul(sq[:], sj[:], sj[:])
                        nc.vector.tensor_reduce(
                            r_[:], sq[:].rearrange("p (m po) -> p m po", po=PO),
                            axis=AXX, op=ALU.add)
                        nc.scalar.add(w_[:], r_[:], 1.0)
                        nc.vector.reciprocal_approx_fast(w_[:], w_[:])
                        nc.scalar.activation(f_[:], r_[:], ACTF.Sqrt)
                        nc.vector.tensor_mul(f_[:], f_[:], w_[:])
                        f_b = f_[:].rearrange(
                            "p (m x) -> p m x", x=1).to_broadcast(
                            (128, NC_ * CO, PO))
                        sjp = sj[:].rearrange("p (m po) -> p m po", po=PO)
                        if t < routings - 1:
                            nc.vector.tensor_mul(
                                vjb[:].rearrange("p (m po) -> p m po", po=PO),
                                sjp, f_b)
                            # b-upd: e *= exp(reduce_po(V * vj_bc_ci))
                            Wk = wk.tile([128, CI * NC_ * OC], BF16,
                                         tag="scr", name="Wk2")
                            nc.vector.tensor_mul(
                                Wk[:].rearrange("p (ci m) -> p ci m", ci=CI),
                                V[:].rearrange("p (ci m) -> p ci m", ci=CI),
                                vjb[:].rearrange("p (x m) -> p x m",
                                                 x=1).to_broadcast(
                                    (128, CI, NC_ * OC)))
                            nc.vector.tensor_reduce(
                                upd[:],
                                Wk[:].rearrange("p (m po) -> p m po", po=PO),
                                axis=AXX, op=ALU.add)
                            if t == 0:
                                nc.scalar.activation(e[:], upd[:], ACTF.Exp)
                            else:
                                nc.scalar.activation(et[:], upd[:], ACTF.Exp)
                                nc.vector.tensor_mul(e[:], e[:], et[:])
                            # softmax over co: cij = e / reduce_co(e)
                            nc.vector.tensor_reduce(
                                D_[:],
                                e[:].rearrange("p (m co) -> p m co", co=CO),
                                axis=AXX, op=ALU.add)
                            nc.vector.reciprocal_approx_fast(rD[:], D_[:])
                            nc.vector.tensor_mul(
                                cij[:].rearrange("p (m co) -> p m co", co=CO),
                                e[:].rearrange("p (m co) -> p m co", co=CO),
                                rD[:].rearrange("p (m x) -> p m x",
                                                x=1).to_broadcast(
                                    (128, CI * NC_, CO)))
                        else:
                            vjf = wk.tile([128, NC_ * OC], F32,
                                          tag="scr", name="vjf")
                            nc.vector.tensor_mul(
                                vjf[:].rearrange("p (m po) -> p m po", po=PO),
                                sjp, f_b)
                            nc.sync.dma_start(
                                out_d.ap()[:, q * NC_ * OC:(q + 1) * NC_ * OC],
                                vjf[:])

    return dram, out_d


_CACHE = {}


def _get_compiled(routings: int, reps: int):
    key = (routings, reps)
    if key not in _CACHE:
        nc = bacc.Bacc("TRN2", target_bir_lowering=False, debug=False,
                       num_devices=NCORES)
        _build(nc, routings, reps)
        nc.compile()
        _CACHE[key] = nc
    return _CACHE[key]


def _in_maps(x, conv_w, biases):
    consts = _host_consts(conv_w, biases)
    xt = _host_x(x)
    maps = []
    for b in range(NCORES):
        m = dict(consts)
        m["xpad_t"] = np.ascontiguousarray(xt[b])
        maps.append(m)
    return maps


def _assemble(res):
    """dev out [128, (q, c, co, po)] -> [B, Co, Po, H, W]."""
    out = np.empty((B, CO, PO, SP), np.float32)
    for b in range(NCORES):
        o = res.results[b]["out"].reshape(128, NQ, NC_, CO, PO)
        # s = q*1024 + c*128 + p
        out[b] = o.transpose(3, 4, 1, 2, 0).reshape(CO, PO, SP)
    return out.reshape(B, CO, PO, H, W)


def kernel(x, conv_w, biases, routings):
    routings = int(routings)
    nc = _get_compiled(routings, reps=1)
    res = bass_utils.run_bass_kernel_spmd(nc, _in_maps(x, conv_w, biases),
                                          core_ids=list(range(NCORES)))
    return _assemble(res)


def measure_hw_time_ns(inputs, reps=8, samples=7):
    """Per-iteration HW time via paired repetition-deltas.

    Runs the 1-rep and reps-rep programs back-to-back (paired samples) so
    slow drift in the dispatch/transfer floor cancels; reports the median
    paired delta divided by (reps-1).
    """
    import time
    routings = int(inputs["routings"])
    maps = _in_maps(inputs["x"], inputs["conv_w"], inputs["biases"])
    nc1 = _get_compiled(routings, reps=1)
    ncR = _get_compiled(routings, reps=reps)

    def run_once(nc):
        t0 = time.perf_counter()
        bass_utils.run_bass_kernel_spmd(nc, maps, core_ids=list(range(NCORES)))
        return time.perf_counter() - t0

    run_once(nc1); run_once(ncR)  # warm both (compile + jit caches)
    t1s, tRs, deltas = [], [], []
    for _ in range(samples):
        a = run_once(nc1)
        b = run_once(ncR)
        t1s.append(a); tRs.append(b); deltas.append(b - a)
    deltas.sort()
    med = deltas[len(deltas) // 2]
    print(f"  raw wall: 1rep {min(t1s)*1e3:.1f} ms, {reps}rep {min(tRs)*1e3:.1f} ms"
          f" (paired-delta median {med*1e3:.1f} ms over {samples})")
    return max(1, int(med / (reps - 1) * 1e9))


# revision 8
# speedup vs baseline: 1.0861x; 1.0861x over previous
"""ConvCaps (shared 3x3 conv + dynamic routing) Trainium2 Bass kernel — v2.

Sharding: data-parallel over batch B=8 -> 8 NeuronCores (1 image/core).

v2 design ("spatial-on-partitions" routing layout), sized for
B,Ci,Pi,Co,Po,K,H,W = 8,8,16,16,16,3,64,64:

  The image is processed in 4 quarters of 1024 pixels (16 rows). Per quarter:

  conv:  im2col patches pa[(tap,pi)=128, (ci,s)=8192] built by 9 shifted
         SBUF->SBUF DMAs from a host-prepared padded input stored as
         xpad_t[(pi,ci)=128, 66*66] (one DMA covers all ci). Votes are
         computed in the standard layout Vt[oc_slab=128, (ci,s)] with
         2 K-tile fp32 matmuls (taps0-7: K=128, tap8: K=16; fp32 avoids
         the separate LDWEIGHTS instruction of the 16-bit path) per
         512-chunk, N=512, accumulating in [128,2048] PSUM tiles,
         evacuated by ScalarE copies (fp32->bf16).

  transpose: one InstDmaTransposeAnt per (quarter, oc-slab) turns
         Vt[oc,(ci,s)] into the routing layout V[p=s%128, (ci, c=s//128,
         oc)] (bf16, 2-byte dtype required by the xbar transpose).

  routing: with spatial on partitions, co/po/ci are all free dims, so
         every reduction/broadcast of the dynamic-routing loop is a
         single big DVE/ACT instruction (tensor_reduce over the
         innermost dim / stride-0 broadcast APs). No matmuls, no
         partition ops:
           sj0   = reduce_ci(V)/16 + bias
           squash: r=reduce_po(sj^2); f=sqrt(r)/(1+r); vj=sj*f
           b-upd: e *= exp(reduce_po(V*vj))
           cij   = e / reduce_co(e)
           sj    = reduce_ci(V*cij) + bias
"""

import sys

sys.path.insert(0, "/opt/trn_rl_repo")

import numpy as np
import ml_dtypes

import concourse.bacc as bacc
import concourse.mybir as mybir
import concourse.tile as tile
from concourse import bass_utils

F32 = mybir.dt.float32
BF16 = mybir.dt.bfloat16
ALU = mybir.AluOpType
ACTF = mybir.ActivationFunctionType
AXX = mybir.AxisListType.X

B, CI, PI, CO, PO, KK = 8, 8, 16, 16, 16, 3
H = W = 64
SP = H * W            # 4096 spatial positions per image
NCORES = 8
HP, WP = H + 2, W + 2  # padded 66x66
NQ = 4                 # quarters per image
SQ = SP // NQ          # 1024 pixels per quarter
RQ = SQ // W           # 16 image rows per quarter
NC_ = SQ // 128        # 8 chunks of 128 pixels per quarter
OC = CO * PO           # 256


def _host_consts(conv_w: np.ndarray, biases: np.ndarray):
    cw = np.asarray(conv_w, np.float32)                       # [256, 16, 3, 3]
    wT = cw.transpose(2, 3, 1, 0).reshape(9 * PI, OC)         # [(tap,pi), oc]
    wTa = wT[:128].astype(np.float32)                         # taps 0-7
    wTb = np.ascontiguousarray(wT[128:]).astype(np.float32)   # tap 8
    bias_rep = np.tile(np.asarray(biases, np.float32).reshape(1, OC), (128, 1))
    return dict(wTa=np.ascontiguousarray(wTa), wTb=wTb,
                bias_rep=np.ascontiguousarray(bias_rep))


def _host_x(x: np.ndarray):
    """xpad_t per core: [(pi,ci)=128, 66*66] bf16."""
    xf = np.asarray(x, np.float32)                            # [B, Ci, Pi, H, W]
    xpad = np.zeros((B, CI, PI, HP, WP), np.float32)
    xpad[:, :, :, 1:H + 1, 1:W + 1] = xf
    xt = xpad.transpose(0, 2, 1, 3, 4).reshape(B, 128, HP * WP)  # (pi, ci)
    return np.ascontiguousarray(xt.astype(np.float32))


def _build(nc, routings: int, reps: int):
    dram = {}
    for name, shape, dt in [
        ("xpad_t", [128, HP * WP], F32),
        ("wTa", [128, OC], F32), ("wTb", [16, OC], F32),
        ("bias_rep", [128, OC], F32),
    ]:
        dram[name] = nc.dram_tensor(name, shape, dt, kind="ExternalInput")
    # out[p, (q, c, co, po)] : spatial s = q*1024 + c*128 + p
    out_d = nc.dram_tensor("out", [128, NQ * NC_ * OC], F32,
                           kind="ExternalOutput")

    with tile.TileContext(nc) as tc:
        with (
            tc.tile_pool(name="const", bufs=1) as cpool,
            tc.tile_pool(name="work", bufs=1) as wk,
            tc.tile_pool(name="stage", bufs=1) as stg,
            tc.tile_pool(name="ps", bufs=1, space="PSUM") as psp,
        ):
            xpad_sb = cpool.tile([128, HP * WP], F32, tag="xpad", name="xpad")
            wTa = cpool.tile([128, OC], F32, tag="wTa", name="wTa")
            wTb = cpool.tile([16, OC], F32, tag="wTb", name="wTb")
            bias_rep = cpool.tile([128, OC], F32, tag="bias", name="bias")
            for nm, t in [("xpad_t", xpad_sb), ("wTa", wTa), ("wTb", wTb),
                          ("bias_rep", bias_rep)]:
                nc.sync.dma_start(t[:], dram[nm].ap())
            xpad3 = xpad_sb[:].rearrange("p (h w) -> p h w", h=HP)

            for _rep in range(reps):
                for q in range(NQ):
                    r0 = q * RQ

                    # ---- im2col patches: one DMA per tap ----
                    pa = stg.tile([128, CI * SQ], F32, tag="pa", name="pa")
                    pb = stg.tile([16, CI * SQ], F32, tag="pb", name="pb")
                    for tap in range(9):
                        dh, dw = tap // 3, tap % 3
                        src = xpad3[:, r0 + dh:r0 + dh + RQ, dw:dw + W]
                        if tap < 8:
                            dst = pa[tap * 16:(tap + 1) * 16].rearrange(
                                "p (ci r c) -> p ci r c", ci=CI, r=RQ)
                        else:
                            dst = pb[:].rearrange(
                                "p (ci r c) -> p ci r c", ci=CI, r=RQ)
                        nc.sync.dma_start(dst, src)

                    # ---- conv -> Vt[oc_slab, (ci,s)] -> transpose to V ----
                    # V free layout: (ci 8, c 8, oc 256)
                    V = wk.tile([128, CI * NC_ * OC], BF16, tag="V", name="V")
                    for slab in range(2):
                        vt = stg.tile([128, CI * SQ], BF16, tag="vt", name="vt")
                        for g in range(2):
                            ps = psp.tile([128, 4096], F32, tag="cps", name="cps")
                            for cc in range(8):
                                ch = g * 8 + cc
                                nc.tensor.matmul(
                                    ps[:, cc * 512:(cc + 1) * 512],
                                    wTa[:, slab * 128:(slab + 1) * 128],
                                    pa[:, ch * 512:(ch + 1) * 512],
                                    start=True, stop=False)
                                nc.tensor.matmul(
                                    ps[:, cc * 512:(cc + 1) * 512],
                                    wTb[:, slab * 128:(slab + 1) * 128],
                                    pb[:, ch * 512:(ch + 1) * 512],
                                    start=False, stop=True)
                            nc.scalar.copy(vt[:, g * 4096:(g + 1) * 4096], ps[:])
                        # rows r=(ci,s_local): k=r//128=(ci,c), p=s_local%128
                        nc.sync.dma_start_transpose(
                            V[:].rearrange("p (k oc) -> p k oc", oc=OC)
                               [:, :, slab * 128:(slab + 1) * 128],
                            vt[:])

                    # ---- routing (all free-dim ops) ----
                    sj = wk.tile([128, NC_ * OC], F32, tag="sj", name="sj")
                    vjb = wk.tile([128, NC_ * OC], BF16, tag="vjb", name="vjb")
                    e = wk.tile([128, CI * NC_ * CO], F32, tag="e", name="e")
                    cij = wk.tile([128, CI * NC_ * CO], BF16, tag="cij", name="cij")
                    sq = wk.tile([128, NC_ * OC], F32, tag="sq", name="sq")
                    upd = wk.tile([128, CI * NC_ * CO], F32, tag="upd", name="upd")
                    et = wk.tile([128, CI * NC_ * CO], F32, tag="et", name="et")
                    r_ = wk.tile([128, NC_ * CO], F32, tag="r_", name="r_")
                    w_ = wk.tile([128, NC_ * CO], F32, tag="w_", name="w_")
                    f_ = wk.tile([128, NC_ * CO], F32, tag="f_", name="f_")
                    D_ = wk.tile([128, CI * NC_], F32, tag="D_", name="D_")
                    rD = wk.tile([128, CI * NC_], F32, tag="rD", name="rD")

                    bias_b = bias_rep[:].rearrange(
                        "p (c m) -> p c m", c=1).to_broadcast((128, NC_, OC))
                    sj3 = sj[:].rearrange("p (c m) -> p c m", c=NC_)

                    for t in range(routings):
                        if t == 0:
                            # sj0 = reduce_ci(V)/16 + bias
                            nc.vector.tensor_reduce(
                                sj[:],
                                V[:].rearrange("p (ci m) -> p m ci", ci=CI),
                                axis=AXX, op=ALU.add)
                            nc.vector.scalar_tensor_tensor(
                                out=sj3, in0=sj3, scalar=1.0 / CO,
                                in1=bias_b, op0=ALU.mult, op1=ALU.add)
                        else:
                            # sj = reduce_ci(V * cij_bc_po) + bias
                            Wk = wk.tile([128, CI * NC_ * OC], BF16,
                                         tag="scr", name="Wk")
                            nc.vector.tensor_mul(
                                Wk[:].rearrange("p (g co po) -> p g co po",
                                                co=CO, po=PO),
                                V[:].rearrange("p (g co po) -> p g co po",
                                               co=CO, po=PO),
                                cij[:].rearrange("p (g co x) -> p g co x", co=CO,
                                                 x=1).to_broadcast(
                                    (128, CI * NC_, CO, PO)))
                            nc.vector.tensor_reduce(
                                sj[:],
                                Wk[:].rearrange("p (ci m) -> p m ci", ci=CI),
                                axis=AXX, op=ALU.add)
                            nc.vector.tensor_add(sj3, sj3, bias_b)

                        # squash: f = sqrt(r)/(1+r), vj = sj*f
                        nc.vector.tensor_mul(sq[:], sj[:], sj[:])
                        nc.vector.tensor_reduce(
                            r_[:], sq[:].rearrange("p (m po) -> p m po", po=PO),
                            axis=AXX, op=ALU.add)
                        nc.scalar.add(w_[:], r_[:], 1.0)
                        nc.vector.reciprocal_approx_fast(w_[:], w_[:])
                        nc.scalar.activation(f_[:], r_[:], ACTF.Sqrt)
                        nc.vector.tensor_mul(f_[:], f_[:], w_[:])
                        f_b = f_[:].rearrange(
                            "p (m x) -> p m x", x=1).to_broadcast(
                            (128, NC_ * CO, PO))
                        sjp = sj[:].rearrange("p (m po) -> p m po", po=PO)
                        if t < routings - 1:
                            nc.vector.tensor_mul(
                                vjb[:].rearrange("p (m po) -> p m po", po=PO),
                                sjp, f_b)
                            # b-upd: e *= exp(reduce_po(V * vj_bc_ci))
                            Wk = wk.tile([128, CI * NC_ * OC], BF16,
                                         tag="scr", name="Wk2")
                            nc.vector.tensor_mul(
                                Wk[:].rearrange("p (ci m) -> p ci m", ci=CI),
                                V[:].rearrange("p (ci m) -> p ci m", ci=CI),
                                vjb[:].rearrange("p (x m) -> p x m",
                                                 x=1).to_broadcast(
                                    (128, CI, NC_ * OC)))
                            nc.vector.tensor_reduce(
                                upd[:],
                                Wk[:].rearrange("p (m po) -> p m po", po=PO),
                                axis=AXX, op=ALU.add)
                            if t == 0:
                                nc.scalar.activation(e[:], upd[:], ACTF.Exp)
                            else:
                                nc.scalar.activation(et[:], upd[:], ACTF.Exp)
                                nc.vector.tensor_mul(e[:], e[:], et[:])
                            # softmax over co: cij = e / reduce_co(e)
                            nc.vector.tensor_reduce(
                                D_[:],
                                e[:].rearrange("p (m co) -> p m co", co=CO),
                                axis=AXX, op=ALU.add)
                            nc.vector.reciprocal_approx_fast(rD[:], D_[:])
                            nc.vector.tensor_mul(
                                cij[:].rearrange("p (m co) -> p m co", co=CO),
                                e[:].rearrange("p (m co) -> p m co", co=CO),
                                rD[:].rearrange("p (m x) -> p m x",
                                                x=1).to_broadcast(
                                    (128, CI * NC_, CO)))
                        else:
                            vjf = wk.tile([128, NC_ * OC], F32,
                                          tag="scr", name="vjf")
                            nc.vector.tensor_mul(
                                vjf[:].rearrange("p (m po) -> p m po", po=PO),
                                sjp, f_b)
                            nc.sync.dma_start(
                                out_d.ap()[:, q * NC_ * OC:(q + 1) * NC_ * OC],
                                vjf[:])

    return dram, out_d


_CACHE = {}


def _get_compiled(routings: int, reps: int):
    key = (routings, reps)
    if key not in _CACHE:
        nc = bacc.Bacc("TRN2", target_bir_lowering=False, debug=False,
                       num_devices=NCORES)
        _build(nc, routings, reps)
        nc.compile()
        _CACHE[key] = nc
    return _CACHE[key]


def _in_maps(x, conv_w, biases):
    consts = _host_consts(conv_w, biases)
    xt = _host_x(x)
    maps = []
    for b in range(NCORES):
        m = dict(consts)
        m["xpad_t"] = np.ascontiguousarray(xt[b])
        maps.append(m)
    return maps


def _assemble(res):
    """dev out [128, (q, c, co, po)] -> [B, Co, Po, H, W]."""
    out = np.empty((B, CO, PO, SP), np.float32)
    for b in range(NCORES):
        o = res.results[b]["out"].reshape(128, NQ, NC_, CO, PO)
        # s = q*1024 + c*128 + p
        out[b] = o.transpose(3, 4, 1, 2, 0).reshape(CO, PO, SP)
    return out.reshape(B, CO, PO, H, W)


def kernel(x, conv_w, biases, routings):
    routings = int(routings)
    nc = _get_compiled(routings, reps=1)
    res = bass_utils.run_bass_kernel_spmd(nc, _in_maps(x, conv_w, biases),
                                          core_ids=list(range(NCORES)))
    return _assemble(res)


def measure_hw_time_ns(inputs, reps=8, samples=7):
    """Per-iteration HW time via paired repetition-deltas.

    Runs the 1-rep and reps-rep programs back-to-back (paired samples) so
    slow drift in the dispatch/transfer floor cancels; reports the median
    paired delta divided by (reps-1).
    """
    import time
    routings = int(inputs["routings"])
    maps = _in_maps(inputs["x"], inputs["conv_w"], inputs["biases"])
    nc1 = _get_compiled(routings, reps=1)
    ncR = _get_compiled(routings, reps=reps)

    def run_once(nc):
        t0 = time.perf_counter()
        bass_utils.run_bass_kernel_spmd(nc, maps, core_ids=list(range(NCORES)))
        return time.perf_counter() - t0

    run_once(nc1); run_once(ncR)  # warm both (compile + jit caches)
    t1s, tRs, deltas = [], [], []
    for _ in range(samples):
        a = run_once(nc1)
        b = run_once(ncR)
        t1s.append(a); tRs.append(b); deltas.append(b - a)
    deltas.sort()
    med = deltas[len(deltas) // 2]
    print(f"  raw wall: 1rep {min(t1s)*1e3:.1f} ms, {reps}rep {min(tRs)*1e3:.1f} ms"
          f" (paired-delta median {med*1e3:.1f} ms over {samples})")
    return max(1, int(med / (reps - 1) * 1e9))
